# revision 5
# baseline (speedup 1.0000x reference)
"""Block-causal attention block (RMSnorm + QKV + frame-causal attention + proj)
on 8 TRN2 NeuronCores — fp8 DoubleRow + weight-folding edition.

Sharding: as the baseline — core j (p=j//2, h=j%2) owns query half-blocks
(frame p, col-half h) and (frame 7-p, col-half h); each core runs a uniform
stream of 18 (kv half-block, q-half) pair-steps (perfectly balanced since
2(p+1) + 2(8-p) = 18).

Algebraic folds (vs the baseline's per-step K/V projections):
  - K-fold: scores s[kv,q] = x_raw[:,kv]^T (Wk_fold q) — the K projection
    moves to the query side (one GEMM per q-half); the k-bias term is
    constant per query row and drops by softmax shift invariance.
  - V-fold: O = Wv_fold (sum_kv xn[:,kv] p[kv,q]) — the V projection moves
    to after the attention sum (one GEMM per q-half); bv folds through wp
    into the output bias since softmax rows sum to 1.
  - Norm-fold: RMS norm scalar rho[t] = sqrt(C)/||x_t|| is computed on the
    DVE (free-axis square-reduce of the transposed slab + Quake rsqrt) and
    applied as (a) the per-partition scale AP of the Exp activation on the
    kv side and (b) a broadcast row multiply on the q side. gamma folds
    into the weights host-side.

All matmuls run in fp8 e4m3 with perf_mode=DoubleRow (K=256 per
instruction, 2 fp8 MACs/cell/cycle). Weights are pre-scaled by 64 (16 for
Wv) host-side to sit in e4m3's normal range; the compensations fold into
the exp scale and two output-side constants. Residual x + bias stays f32.

Per-pair-step PE work: 8 DR matmuls (scores) + 8 (U accum) + 2 (den)
vs the baseline's 64+8 full-rate f32r matmuls per step.
"""

import sys

import numpy as np
import ml_dtypes

sys.path.insert(0, "/opt/trn_rl_repo")

import concourse.bacc as bacc
import concourse.bass as bass
import concourse.tile as tile
from concourse import mybir
from concourse.bass_utils import run_bass_kernel_spmd

C = 512
CC = C // 128          # 4 chunks of 128
F = 8                  # frames
HW = 1024              # tokens per frame
S = 512                # tokens per half-block / step
NSTEP = 18             # pair-steps per core (balanced)
Q = 1024               # queries per core (two half-blocks)
SW = 64.0              # Wq/Wk host scale
SWV = 16.0             # Wv host scale
SWP = 64.0             # Wp host scale
SU = 64.0              # U quantize scale
SQC = float(np.sqrt(C))
SCALE = 1.0 / SQC
MAGIC = 0x5F3759DF     # Quake rsqrt seed

F32 = mybir.dt.float32
F32R = mybir.dt.float32r
F8 = mybir.dt.float8e4
I32 = mybir.dt.int32
Act = mybir.ActivationFunctionType
Alu = mybir.AluOpType
DR = mybir.MatmulPerfMode.DoubleRow
E4NP = ml_dtypes.float8_e4m3

_cached = {}


def _build():
    if "nc" in _cached:
        return _cached["nc"]

    nc = bacc.Bacc()
    x8_d = nc.dram_tensor("x8", [C, NSTEP * S], F8, kind="ExternalInput")
    xT8_d = nc.dram_tensor("xT8", [NSTEP * S, C], F8, kind="ExternalInput")
    qoff_d = nc.dram_tensor("qoff", [1, NSTEP], I32, kind="ExternalInput")
    wq8_d = nc.dram_tensor("wq8", [C, C], F8, kind="ExternalInput")
    wk8_d = nc.dram_tensor("wk8", [C, C], F8, kind="ExternalInput")
    wv8_d = nc.dram_tensor("wv8", [C, C], F8, kind="ExternalInput")
    wp8_d = nc.dram_tensor("wp8", [C, C], F8, kind="ExternalInput")
    cvec_d = nc.dram_tensor("cvec", [C, 1], F32, kind="ExternalInput")
    xres_d = nc.dram_tensor("xres", [C, Q], F32, kind="ExternalInput")
    ident_d = nc.dram_tensor("ident", [128, 128], F32, kind="ExternalInput")
    out_d = nc.dram_tensor("out", [C, Q], F32, kind="ExternalOutput")

    with tile.TileContext(nc) as tc:
        with (
            tc.tile_pool(name="const", bufs=1) as const,
            tc.tile_pool(name="persist", bufs=1) as persist,
            tc.tile_pool(name="stream", bufs=4) as stream,
            tc.tile_pool(name="small", bufs=2) as small,
            tc.tile_pool(name="psum_sc", bufs=2, space="PSUM") as psum_sc,
            tc.tile_pool(name="psum_gen", bufs=2, space="PSUM") as psum_gen,
            tc.tile_pool(name="psum_den", bufs=2, space="PSUM") as psum_den,
        ):
            # ---- constants / weights ----
            wq8_sb = const.tile([128, CC, C], F8, tag="wq8", name="wq8_sb")
            wk8_sb = const.tile([128, CC, C], F8, tag="wk8", name="wk8_sb")
            wv8_sb = const.tile([128, CC, C], F8, tag="wv8", name="wv8_sb")
            wp8_sb = const.tile([128, CC, C], F8, tag="wp8", name="wp8_sb")
            for w_sb, w_d in (
                (wq8_sb, wq8_d), (wk8_sb, wk8_d), (wv8_sb, wv8_d), (wp8_sb, wp8_d),
            ):
                for ci in range(CC):
                    nc.sync.dma_start(
                        out=w_sb[:, ci, :], in_=w_d[ci * 128:(ci + 1) * 128, :],
                    )
            cvec_sb = const.tile([128, CC], F32, tag="cvec", name="cvec_sb")
            for ci in range(CC):
                nc.sync.dma_start(
                    out=cvec_sb[:, ci:ci + 1],
                    in_=cvec_d[ci * 128:(ci + 1) * 128, :],
                )
            ident_sb = const.tile([128, 128], F32R, tag="ident", name="ident_sb")
            nc.sync.dma_start(out=ident_sb[:], in_=ident_d[:].bitcast(F32R))
            qoff_sb = const.tile([1, NSTEP], I32, tag="qoff", name="qoff_sb")
            nc.sync.dma_start(out=qoff_sb[:], in_=qoff_d[:])
            xres_sb = const.tile([128, CC, Q], F32, tag="xres", name="xres_sb")
            for ci in range(CC):
                nc.sync.dma_start(
                    out=xres_sb[:, ci, :], in_=xres_d[ci * 128:(ci + 1) * 128, :],
                )
            ones8 = const.tile([128, CC, 16], F8, tag="ones8", name="ones8")
            nc.vector.memset(ones8[:], 1.0)
            invc_sb = const.tile([128, 1], F32, tag="invc", name="invc_sb")
            nc.vector.memset(invc_sb[:], 1.0 / (SWP * SWV * SU))

            # ---- persistent accumulators ----
            qk8_sb = persist.tile([128, CC, Q], F8, tag="qk8", name="qk8_sb")
            U_sb = persist.tile([128, CC, Q], F32, tag="U", name="U_sb")
            nc.vector.memset(U_sb[:], 0.0)
            den_sb = persist.tile([1, Q], F32, tag="den", name="den_sb")
            nc.vector.memset(den_sb[:], 0.0)

            # ---- PE warmup: ~4.3us of back-to-back matmuls opens the HAM
            # clock gate (4/8 -> 8/8) before the real stream begins ----
            ones_f = const.tile([128, 1], F32, tag="ones_f", name="ones_f")
            nc.vector.memset(ones_f[:], 1.0)
            ones_r = const.tile([128, 1], F32R, tag="ones_r", name="ones_r")
            nc.vector.tensor_copy(ones_r[:], ones_f[:])
            warm_f = small.tile([128, S], F32, tag="warmf", name="warm_f", bufs=1)
            nc.vector.memset(warm_f[:], 0.0)
            warm_r = small.tile([128, S], F32R, tag="warmr", name="warm_r", bufs=1)
            nc.vector.tensor_copy(warm_r[:], warm_f[:])
            warm_ps = psum_den.tile([1, S], F32, tag="den", name="warm_ps")
            for wi in range(20):
                nc.tensor.matmul(
                    warm_ps[:], ones_r[:], warm_r[:],
                    start=(wi == 0), stop=(wi == 19),
                )

            x8s = {}
            xT8s = {}
            rhoTs = {}
            scexps = {}
            xnT8s = {}

            def load_step(i):
                x8t = stream.tile([128, CC, S], F8, tag="x8", name="x8t")
                for ci in range(CC):
                    nc.sync.dma_start(
                        out=x8t[:, ci, :],
                        in_=x8_d[ci * 128:(ci + 1) * 128, i * S:(i + 1) * S],
                    )
                xT8t = stream.tile([128, CC, C], F8, tag="xT8", name="xT8t")
                for kp in range(CC):
                    nc.sync.dma_start(
                        out=xT8t[:, kp, :],
                        in_=xT8_d[i * S + kp * 128:i * S + (kp + 1) * 128, :],
                    )
                x8s[i] = x8t
                xT8s[i] = xT8t

            def stats_step(i):
                xT8t = xT8s[i]
                ss = small.tile([128, CC], F32, tag="ss", name="ss", bufs=3)
                # NB: the fused tensor_tensor_reduce crashes TRN2 hardware
                # (NRT_EXEC_UNIT_UNRECOVERABLE) — keep square and reduce split.
                scr = small.tile([128, CC, C], F8, tag="scr", name="scr", bufs=2)
                nc.vector.tensor_mul(scr[:], xT8t[:], xT8t[:])
                nc.vector.tensor_reduce(
                    ss[:], scr[:], axis=mybir.AxisListType.X, op=Alu.add,
                )
                # Quake rsqrt (1 Newton iteration, ~0.2% max rel err)
                yi = small.tile([128, CC], I32, tag="qi1", name="yi")
                nc.vector.tensor_scalar(
                    out=yi[:], in0=ss[:].bitcast(I32),
                    scalar1=1, scalar2=None, op0=Alu.arith_shift_right,
                )
                r0i = small.tile([128, CC], I32, tag="qi2", name="r0i")
                nc.vector.tensor_scalar(
                    out=r0i[:], in0=yi[:],
                    scalar1=-1, scalar2=MAGIC, op0=Alu.mult, op1=Alu.add,
                )
                t1 = small.tile([128, CC], F32, tag="qf1", name="t1")
                nc.vector.tensor_mul(t1[:], ss[:], r0i[:].bitcast(F32))
                t2 = small.tile([128, CC], F32, tag="qf2", name="t2")
                nc.vector.tensor_mul(t2[:], t1[:], r0i[:].bitcast(F32))
                u = small.tile([128, CC], F32, tag="qf3", name="u")
                nc.vector.tensor_scalar(
                    out=u[:], in0=t2[:],
                    scalar1=-0.5, scalar2=1.5, op0=Alu.mult, op1=Alu.add,
                )
                rT = small.tile([128, CC], F32, tag="qf4", name="rT", bufs=3)
                nc.vector.tensor_mul(rT[:], r0i[:].bitcast(F32), u[:])
                rhoT = stream.tile([128, CC], F32, tag="rhoT", name="rhoT")
                nc.vector.tensor_scalar_mul(rhoT[:], rT[:], SQC)
                scexp = stream.tile([128, CC], F32, tag="scexp", name="scexp")
                nc.vector.tensor_scalar_mul(scexp[:], rT[:], 1.0 / SW)
                rhoTs[i] = rhoT
                scexps[i] = scexp

            def xnt_step(i):
                xnT8t = stream.tile([128, CC, C], F8, tag="xnT8", name="xnT8t", bufs=3)
                for kp in range(CC):
                    nc.vector.tensor_scalar_mul(
                        xnT8t[:, kp, :], xT8s[i][:, kp, :], rhoTs[i][:, kp:kp + 1],
                    )
                xnT8s[i] = xnT8t

            def qprep(half, i):
                # rho row for the q tokens: PE mini-transpose of rhoT cols
                rhoR = small.tile([128, CC], F32R, tag="rhoR", name="rhoR")
                nc.vector.tensor_copy(rhoR[:], rhoTs[i][:])
                row_ps = psum_den.tile([1, S], F32, tag="den", name="row_ps")
                for kp in range(CC):
                    nc.tensor.matmul(
                        row_ps[0:1, kp * 128:(kp + 1) * 128],
                        rhoR[:, kp:kp + 1],
                        ident_sb[:],
                        start=True, stop=True,
                    )
                rho_row = small.tile([1, S], F32, tag="rrow", name="rho_row")
                nc.vector.tensor_scalar_mul(rho_row[:], row_ps[:], 1.0 / SW)
                rho_b = small.tile([128, S], F32, tag="rhob", name="rho_b")
                nc.gpsimd.partition_broadcast(rho_b[:], rho_row[:])
                qn8 = small.tile([128, CC, S], F8, tag="qn8", name="qn8")
                for co in range(CC):
                    q0_ps = psum_gen.tile([128, S], F32, tag="gen", name="q0_ps")
                    for t in range(2):
                        nc.tensor.matmul(
                            q0_ps[:],
                            wq8_sb[:, 2 * t:2 * t + 2, co * 128:(co + 1) * 128],
                            x8s[i][:, 2 * t:2 * t + 2, :],
                            start=(t == 0), stop=(t == 1), perf_mode=DR,
                        )
                    nc.vector.tensor_mul(qn8[:, co, :], q0_ps[:], rho_b[:])
                for ci in range(CC):
                    qk_ps = psum_gen.tile([128, S], F32, tag="gen", name="qk_ps")
                    for t in range(2):
                        nc.tensor.matmul(
                            qk_ps[:],
                            wk8_sb[:, 2 * t:2 * t + 2, ci * 128:(ci + 1) * 128],
                            qn8[:, 2 * t:2 * t + 2, :],
                            start=(t == 0), stop=(t == 1), perf_mode=DR,
                        )
                    nc.vector.tensor_scalar_add(
                        qk8_sb[:, ci, half * S:(half + 1) * S],
                        qk_ps[:], cvec_sb[:, ci:ci + 1],
                    )

            def pair_step(i):
                off = nc.values_load(
                    qoff_sb[0:1, i:i + 1],
                    engines=[mybir.EngineType.DVE],
                    min_val=0, max_val=S,
                    skip_runtime_bounds_check=True,
                )
                # the dual-fp8 ISA check rejects register offsets on the
                # matmul rhs, so materialize this step's q-half of qk with a
                # DVE copy (register offsets are fine there)
                qkc = stream.tile([128, CC, S], F8, tag="qkc", name="qkc", bufs=3)
                nc.vector.tensor_copy(qkc[:], qk8_sb[:, :, bass.ds(off, S)])
                p8t = stream.tile([128, CC, S], F8, tag="p8", name="p8t", bufs=3)
                for kh in range(2):
                    s_ps = psum_sc.tile([128, 2, S], F32, tag="sc", name="s_ps")
                    for kp2 in range(2):
                        kp = kh * 2 + kp2
                        for t in range(2):
                            nc.tensor.matmul(
                                s_ps[:, kp2, :],
                                x8s[i][:, 2 * t:2 * t + 2, kp * 128:(kp + 1) * 128],
                                qkc[:, 2 * t:2 * t + 2, :],
                                start=(t == 0), stop=(t == 1), perf_mode=DR,
                            )
                        nc.scalar.activation(
                            p8t[:, kp, :], s_ps[:, kp2, :], Act.Exp,
                            bias=0.0, scale=scexps[i][:, kp:kp + 1],
                        )
                dn_ps = psum_den.tile([1, S], F32, tag="den", name="dn_ps")
                for t in range(2):
                    nc.tensor.matmul(
                        dn_ps[:],
                        ones8[:, 2 * t:2 * t + 2, 0:1],
                        p8t[:, 2 * t:2 * t + 2, :],
                        start=(t == 0), stop=(t == 1), perf_mode=DR,
                    )
                nc.vector.tensor_add(
                    den_sb[:, bass.ds(off, S)], den_sb[:, bass.ds(off, S)], dn_ps[:],
                )
                for ci in range(CC):
                    u_ps = psum_gen.tile([128, S], F32, tag="gen", name="u_ps")
                    for t in range(2):
                        nc.tensor.matmul(
                            u_ps[:],
                            xnT8s[i][:, 2 * t:2 * t + 2, ci * 128:(ci + 1) * 128],
                            p8t[:, 2 * t:2 * t + 2, :],
                            start=(t == 0), stop=(t == 1), perf_mode=DR,
                        )
                    nc.vector.tensor_add(
                        U_sb[:, ci, bass.ds(off, S)],
                        U_sb[:, ci, bass.ds(off, S)],
                        u_ps[:],
                    )

            def finalize(half):
                cols = half * S
                dent = small.tile([1, S], F32, tag="rrow", name="dent")
                nc.vector.tensor_scalar_mul(dent[:], den_sb[:, cols:cols + S], 1.0 / SU)
                rd = small.tile([1, S], F32, tag="rd", name="rd")
                nc.vector.reciprocal(rd[:], dent[:])
                rdb = small.tile([128, S], F32, tag="rhob", name="rdb")
                nc.gpsimd.partition_broadcast(rdb[:], rd[:])
                u8 = small.tile([128, CC, S], F8, tag="u8", name="u8")
                for ci in range(CC):
                    nc.vector.tensor_mul(u8[:, ci, :], U_sb[:, ci, cols:cols + S], rdb[:])
                o8 = small.tile([128, CC, S], F8, tag="o8", name="o8")
                for co in range(CC):
                    ot_ps = psum_gen.tile([128, S], F32, tag="gen", name="ot_ps")
                    for t in range(2):
                        nc.tensor.matmul(
                            ot_ps[:],
                            wv8_sb[:, 2 * t:2 * t + 2, co * 128:(co + 1) * 128],
                            u8[:, 2 * t:2 * t + 2, :],
                            start=(t == 0), stop=(t == 1), perf_mode=DR,
                        )
                    nc.vector.tensor_copy(o8[:, co, :], ot_ps[:])
                for co in range(CC):
                    pr_ps = psum_gen.tile([128, S], F32, tag="gen", name="pr_ps")
                    for t in range(2):
                        nc.tensor.matmul(
                            pr_ps[:],
                            wp8_sb[:, 2 * t:2 * t + 2, co * 128:(co + 1) * 128],
                            o8[:, 2 * t:2 * t + 2, :],
                            start=(t == 0), stop=(t == 1), perf_mode=DR,
                        )
                    res = small.tile([128, S], F32, tag="res", name="res")
                    nc.vector.scalar_tensor_tensor(
                        out=res[:],
                        in0=pr_ps[:],
                        scalar=invc_sb[:],
                        in1=xres_sb[:, co, cols:cols + S],
                        op0=Alu.mult,
                        op1=Alu.add,
                    )
                    nc.sync.dma_start(
                        out=out_d[co * 128:(co + 1) * 128, cols:cols + S], in_=res[:],
                    )

            # ---- schedule ----
            load_step(0)
            stats_step(0)
            load_step(1)
            stats_step(1)
            qprep(0, 0)
            qprep(1, 1)
            for i in range(NSTEP):
                if i + 2 < NSTEP:
                    load_step(i + 2)
                    stats_step(i + 2)
                xnt_step(i)
                pair_step(i)
            finalize(0)
            finalize(1)

    nc.finalize()
    _cached["nc"] = nc
    return nc


def _q8(a):
    a = np.clip(np.asarray(a, np.float32), -240.0, 240.0)
    return a.astype(E4NP)


def _prep_inputs(x, gamma, wq, bq, wk, bk, wv, bv, wp, bp):
    x = np.asarray(x, np.float32)
    X = np.ascontiguousarray(x[0].reshape(C, F * HW))
    g = np.asarray(gamma, np.float32)
    wq = np.asarray(wq, np.float32)
    wk = np.asarray(wk, np.float32)
    wv = np.asarray(wv, np.float32)
    wp = np.asarray(wp, np.float32)
    bq = np.asarray(bq, np.float32)
    bv = np.asarray(bv, np.float32)
    bp = np.asarray(bp, np.float32)

    wq8 = _q8(SW * (wq * g[None, :]).T)      # [cin, o]
    wk8 = _q8(SW * (wk * g[None, :]))        # [o, cin]
    wv8 = _q8(SWV * (wv * g[None, :]).T)     # [cin, o']
    wp8 = _q8(SWP * wp.T)                    # [o', co]
    cvec = (SW * (wk * g[None, :]).T @ np.asarray(bq, np.float32)).astype(np.float32)
    bvp = (bp + wp @ bv).astype(np.float32)

    X8 = _q8(X)                              # [C, seq] fp8
    XT8 = np.ascontiguousarray(X8.T)         # [seq, C] fp8
    ident = np.eye(128, dtype=np.float32)

    common = {
        "wq8": wq8, "wk8": wk8, "wv8": wv8, "wp8": wp8,
        "cvec": np.ascontiguousarray(cvec[:, None]),
        "ident": ident,
    }
    in_maps = []
    for j in range(F):
        p, h = j // 2, j % 2
        fa, fb = p, F - 1 - p
        ba, bb = 2 * fa + h, 2 * fb + h
        steps = [ba, bb]
        steps += [b for b in range(2 * fa + 2) if b != ba]
        steps += [b for b in range(2 * fb + 2) if b != bb]
        assert len(steps) == NSTEP
        qoffs = [0, S] + [0] * (2 * fa + 1) + [S] * (2 * fb + 1)
        m = dict(common)
        m["x8"] = np.ascontiguousarray(
            np.concatenate([X8[:, b * S:(b + 1) * S] for b in steps], axis=1)
        )
        m["xT8"] = np.ascontiguousarray(
            np.concatenate([XT8[b * S:(b + 1) * S, :] for b in steps], axis=0)
        )
        m["qoff"] = np.asarray([qoffs], np.int32)
        xres = np.concatenate(
            [X[:, ba * S:(ba + 1) * S], X[:, bb * S:(bb + 1) * S]], axis=1
        ) + bvp[:, None]
        m["xres"] = np.ascontiguousarray(xres.astype(np.float32))
        in_maps.append(m)
    return in_maps


def kernel(x, gamma, wq, bq, wk, bk, wv, bv, wp, bp, _trace=False):
    nc = _build()
    in_maps = _prep_inputs(x, gamma, wq, bq, wk, bk, wv, bv, wp, bp)
    kwargs = {}
    if _trace:
        kwargs = dict(trace=True, trace_cores=list(range(F)))
    r = run_bass_kernel_spmd(nc, in_maps, core_ids=list(range(F)), **kwargs)
    out = np.empty((1, C, F, HW), np.float32)
    for j in range(F):
        p, h = j // 2, j % 2
        fa, fb = p, F - 1 - p
        res = r.results[j]["out"]
        out[0, :, fa, h * S:h * S + S] = res[:, 0:S]
        out[0, :, fb, h * S:h * S + S] = res[:, S:Q]
    out = out.reshape(1, C, F, 32, 32)
    kernel._last_results = r
    return out
